# revision 1
# baseline (speedup 1.0000x reference)
"""BART attention (B=4, S=2048, D=1024, H=16) on 8 Trainium2 NeuronCores.

Sharding: tensor-parallel across heads.  Core c owns heads {2c, 2c+1}, i.e.
projection output dims [128c, 128c+128) of wq/wk/wv and rows [128c, 128c+128)
of wo.  Each core computes its two heads' attention over the full batch and a
partial output projection; the host sums the 8 partial outputs.

Device layout per core (all matmuls in float32r: full PE speed, ~1e-4 rel):
  qT, kT  [128 head-dims, 8192 tokens]   (transposed projections)
  v_comb  [tokens, 130] = [vA(64) | 1 | vB(64) | 1]  (ones col -> softmax sums)
  scoresT [k-tok, q-tok] per (batch, head): softmax denom = extra out row of
  the ones-augmented attn@v matmul; exp on ScalarE with fused 1/8 scale; the
  1/sum normalization is applied after attn@v (flash-attention style).
"""
import numpy as np

import concourse.bass as bass
import concourse.mybir as mybir
import concourse.tile as tile
from concourse.bass_utils import run_bass_kernel_spmd
from concourse.masks import make_identity
from concourse.vector_clock import ScopedClock

F32 = mybir.dt.float32
F32R = mybir.dt.float32r
EXPF = mybir.ActivationFunctionType.Exp

B, S, D = 4, 2048, 1024
T = B * S                      # 8192 tokens
NCORES = 8
P = 128                        # partitions / head-dims per core
DK = 64                        # head dim
KC = D // P                    # 8 contraction chunks for projections
TCH = 512                      # token chunk (projection N / q-chunk)
NTCH = T // TCH                # 16
VW = 2 * DK + 2                # 130: [vA | 1 | vB | 1]

# ---------------------------------------------------------------------------
# walrus in this toolchain encodes at most ONE sync wait per instruction
# (two on EventSemaphore).  Tile emits more.  Legalize by carrying excess
# waits on same-engine NOPs inserted right before the instruction (engines
# execute in order, so this is equivalent), and by splitting the kernel-tail
# drain's global-clock waits across a chain of drains.
# ---------------------------------------------------------------------------
_split_counter = [0]


def _legalize_waits(nc):
    inserted = 0
    for fn in nc.m.functions:
        for bb in fn.blocks:
            new_insts = []
            changed = False
            for inst in bb.instructions:
                si = inst.sync_info
                waits = list(si.on_wait) if si is not None and si.on_wait else []
                cap = 2 if inst.opcode == "EventSemaphore" else 1
                if len(waits) > cap:
                    excess, keep = waits[:-cap], waits[-cap:]
                    for w in excess:
                        _split_counter[0] += 1
                        nop = mybir.InstNoOp(
                            name=f"I-waitsplit-{_split_counter[0]}", ins=[], outs=[]
                        )
                        nop.engine = inst.engine
                        nop.sync_info = mybir.SyncInfo(on_wait=[w], on_update=[])
                        new_insts.append(nop)
                        inserted += 1
                    si.on_wait = keep
                    changed = True
                new_insts.append(inst)
            if changed:
                bb.instructions.clear()
                for i in new_insts:
                    bb.instructions.append(i)
    return inserted


class _TC(tile.TileContext):
    def _drain_and_barrier(self, tick_clock, wait_clock):
        drain_inst = self.nc.sync.drain()
        wait_clock.add_sem_waits(
            drain_inst.ins, ScopedClock({None: tick_clock.global_clock})
        )
        si = drain_inst.ins.sync_info
        waits = list(si.on_wait or []) if si is not None else []
        if len(waits) > 1:
            si.on_wait = [waits[0]]
            for w in waits[1:]:
                d = self.nc.sync.drain()
                dsi = d.ins.sync_info
                if dsi is None:
                    d.ins.sync_info = mybir.SyncInfo(on_wait=[w], on_update=[])
                else:
                    dsi.on_wait = [w]
        self.nc.all_engine_barrier()
        assert self.sems is not None
        popped = self.nc._tile_sem_poison_stack.pop()
        assert popped is self._sem_poison
        self.nc.clear_and_free_semaphores(list(self.sems.allocated().values()))
        self.nc.all_engine_barrier()


# ---------------------------------------------------------------------------
# device program (identical on all 8 cores; only input data differs)
# ---------------------------------------------------------------------------
def _build_nc(repeat=1):
    nc = bass.Bass("TRN2", target_bir_lowering=False, debug=False,
                   num_devices=NCORES)
    xt = nc.dram_tensor("xt", [D, T], F32R, kind="ExternalInput").ap()
    wqm = nc.dram_tensor("wqm", [D, P], F32R, kind="ExternalInput").ap()
    wqb = nc.dram_tensor("wqb", [1, P], F32R, kind="ExternalInput").ap()
    wkm = nc.dram_tensor("wkm", [D, P], F32R, kind="ExternalInput").ap()
    wkb = nc.dram_tensor("wkb", [1, P], F32R, kind="ExternalInput").ap()
    wvm = nc.dram_tensor("wvm", [D, P], F32R, kind="ExternalInput").ap()
    wvb = nc.dram_tensor("wvb", [1, P], F32R, kind="ExternalInput").ap()
    wot = nc.dram_tensor("wo", [P, D], F32R, kind="ExternalInput").ap()
    bot = nc.dram_tensor("bo", [KC, P], F32, kind="ExternalInput").ap()
    yt = nc.dram_tensor("yt", [D, T], F32, kind="ExternalOutput").ap()

    with _TC(nc) as tc, nc.allow_low_precision(
            reason="float32r is 32-bit; PE rounds internally"):
        _emit(nc, tc, xt, wqm, wqb, wkm, wkb, wvm, wvb, wot, bot, yt,
              repeat=repeat)
    n = _legalize_waits(nc)
    return nc, n


def _emit(nc, tc, xt, wqm, wqb, wkm, wkb, wvm, wvb, wot, bot, yt, repeat=1):
    ctxs = []

    def pool(name, bufs, space="SBUF"):
        p = tc.tile_pool(name=name, bufs=bufs, space=space)
        ctxs.append(p)
        return p.__enter__()

    wpool = pool("w", 1)
    persist = pool("persist", 1)
    xpool = pool("x", 2)
    scrpool = pool("scr", 2)
    epool = pool("e", 3)
    orawpool = pool("oraw", 2)
    sumpool = pool("sums", 2)
    stgpool = pool("stg", 2)
    ystpool = pool("yst", 2)
    spool = pool("ps_s", 2, space="PSUM")     # [128,1024] = 2 banks/slot
    opool = pool("ps_o", 2, space="PSUM")     # 1 bank/slot
    ypool = pool("ps_y", 2, space="PSUM")     # 1 bank/slot

    # ---- constants / weights (loaded once) ----
    wq_sb = wpool.tile([P, KC, P], F32R)
    wk_sb = wpool.tile([P, KC, P], F32R)
    wv_sb = wpool.tile([P, KC, P], F32R)
    nc.sync.dma_start(wq_sb[:], wqm.rearrange("(k p) d -> p k d", p=P))
    nc.sync.dma_start(wk_sb[:], wkm.rearrange("(k p) d -> p k d", p=P))
    nc.sync.dma_start(wv_sb[:], wvm.rearrange("(k p) d -> p k d", p=P))
    wqb_sb = wpool.tile([1, P], F32R)
    wkb_sb = wpool.tile([1, P], F32R)
    wvb_sb = wpool.tile([1, P], F32R)
    nc.sync.dma_start(wqb_sb[:], wqb[0:1, :])
    nc.sync.dma_start(wkb_sb[:], wkb[0:1, :])
    nc.sync.dma_start(wvb_sb[:], wvb[0:1, :])
    wo_sb = wpool.tile([P, D], F32R)
    nc.sync.dma_start(wo_sb[:], wot[:, :])
    bo_sb = wpool.tile([P, KC], F32)
    nc.sync.dma_start(bo_sb[:], bot.rearrange("m p -> p m"))
    # memset can't write float32r; memset f32 then DVE-copy (which rounds)
    ones_f32 = wpool.tile([P, TCH], F32)
    nc.vector.memset(ones_f32[:], 1.0)
    ones_sb = wpool.tile([1, TCH], F32R)
    nc.vector.tensor_copy(ones_sb[:], ones_f32[0:1, :])
    ident = wpool.tile([P, P], F32)
    make_identity(nc, ident[:])

    # ---- persistent activations ----
    qT = persist.tile([P, T], F32R)
    kT = persist.tile([P, T], F32R)
    v_comb = persist.tile([P, T // P, VW], F32R)    # [tok%128, tok-tile, 130]
    nc.vector.tensor_copy(
        v_comb[:, :, DK:DK + 1],
        ones_f32[:, 0:1].broadcast_to([P, T // P, 1]))
    nc.vector.tensor_copy(
        v_comb[:, :, VW - 1:VW],
        ones_f32[:, 0:1].broadcast_to([P, T // P, 1]))

    NQC = S // TCH                # 4 q-chunks per batch
    NKT = S // P                  # 16 k-tiles per batch

    for b in [b for _ in range(repeat) for b in range(B)]:
        t0 = b * S
        # ================= phase P: q/k/v projections for batch b ==========
        for i in range(S // TCH):
            c0 = t0 + i * TCH
            x_ch = xpool.tile([P, KC, TCH], F32R, tag="x")
            nc.sync.dma_start(
                x_ch[:], xt[:, c0:c0 + TCH].rearrange("(k p) n -> p k n", p=P))
            s_t = spool.tile([P, 2 * TCH], F32, tag="s")
            v_ps = ypool.tile([P, TCH], F32, tag="y")
            for kc in range(KC):
                st = kc == 0
                nc.tensor.matmul(s_t[:, 0:TCH], wq_sb[:, kc, :], x_ch[:, kc, :],
                                 start=st, stop=False)
                nc.tensor.matmul(s_t[:, TCH:2 * TCH], wk_sb[:, kc, :],
                                 x_ch[:, kc, :], start=st, stop=False)
                nc.tensor.matmul(v_ps[:], wv_sb[:, kc, :], x_ch[:, kc, :],
                                 start=st, stop=False)
            nc.tensor.matmul(s_t[:, 0:TCH], wqb_sb[:], ones_sb[:],
                             start=False, stop=True)
            nc.tensor.matmul(s_t[:, TCH:2 * TCH], wkb_sb[:], ones_sb[:],
                             start=False, stop=True)
            nc.tensor.matmul(v_ps[:], wvb_sb[:], ones_sb[:],
                             start=False, stop=True)
            nc.vector.tensor_copy(qT[:, c0:c0 + TCH], s_t[:, 0:TCH])
            nc.vector.tensor_copy(kT[:, c0:c0 + TCH], s_t[:, TCH:2 * TCH])
            v_scr = scrpool.tile([P, TCH], F32, tag="vscr")
            nc.vector.tensor_copy(v_scr[:], v_ps[:])
            for tt in range(TCH // P):
                vt = (c0 // P) + tt
                tr = opool.tile([P, TCH], F32, tag="o")
                nc.tensor.transpose(tr[:, 0:P], v_scr[:, tt * P:(tt + 1) * P],
                                    ident[:])
                nc.vector.tensor_copy(v_comb[:, vt, 0:DK], tr[:, 0:DK])
                nc.vector.tensor_copy(v_comb[:, vt, DK + 1:2 * DK + 1],
                                      tr[:, DK:2 * DK])

        # ================= phase A: attention for batch b ==================
        sums_pp = sumpool.tile([2 * NQC, TCH], F32, tag="sumpp")
        oraw = orawpool.tile([P, S], F32R, tag="oraw")
        for qc in range(NQC):
            q0 = t0 + qc * TCH
            ps_oA = opool.tile([DK + 1, TCH], F32, tag="o")
            ps_oB = opool.tile([DK + 1, TCH], F32, tag="o")
            # software pipeline: attn@v for kc runs one step behind the
            # scores/exp of kc+1 so the PE never serializes behind ACT.
            def attnv(kc, e_t):
                vt = (t0 // P) + kc
                nc.tensor.matmul(ps_oA[:], v_comb[:, vt, 0:DK + 1],
                                 e_t[:, 0:TCH],
                                 start=(kc == 0), stop=(kc == NKT - 1))
                nc.tensor.matmul(ps_oB[:], v_comb[:, vt, DK + 1:VW],
                                 e_t[:, TCH:2 * TCH],
                                 start=(kc == 0), stop=(kc == NKT - 1))

            pending = None
            for kc in range(NKT):
                kt0 = t0 + kc * P
                s_t = spool.tile([P, 2 * TCH], F32, tag="s")
                nc.tensor.matmul(s_t[:, 0:TCH], kT[0:DK, kt0:kt0 + P],
                                 qT[0:DK, q0:q0 + TCH], start=True, stop=True)
                nc.tensor.matmul(s_t[:, TCH:2 * TCH], kT[DK:P, kt0:kt0 + P],
                                 qT[DK:P, q0:q0 + TCH], start=True, stop=True)
                e_t = epool.tile([P, 2 * TCH], F32R, tag="e")
                nc.scalar.activation(e_t[:], s_t[:], EXPF, scale=0.125)
                if pending is not None:
                    attnv(*pending)
                pending = (kc, e_t)
            attnv(*pending)
            # stash softmax denominators (row DK) and raw outputs.
            # DVE writes must start at a 32-aligned partition, so stage each
            # sums row at partition 0 and DMA it to its sums_pp row.
            s_stgA = stgpool.tile([1, TCH], F32, tag="sstg")
            nc.vector.tensor_copy(s_stgA[:], ps_oA[DK:DK + 1, :])
            nc.sync.dma_start(sums_pp[2 * qc:2 * qc + 1, :], s_stgA[:])
            s_stgB = stgpool.tile([1, TCH], F32, tag="sstg")
            nc.vector.tensor_copy(s_stgB[:], ps_oB[DK:DK + 1, :])
            nc.sync.dma_start(sums_pp[2 * qc + 1:2 * qc + 2, :], s_stgB[:])
            nc.vector.tensor_copy(oraw[0:DK, qc * TCH:(qc + 1) * TCH],
                                  ps_oA[0:DK, :])
            nc.vector.tensor_copy(oraw[DK:P, qc * TCH:(qc + 1) * TCH],
                                  ps_oB[0:DK, :])
        # normalization: r = 1/sums, broadcast over 64 partitions, multiply
        recip_pp = sumpool.tile([2 * NQC, TCH], F32R, tag="recip")
        nc.vector.reciprocal(recip_pp[:], sums_pp[:])
        for qc in range(NQC):
            for h in range(2):
                r = 2 * qc + h
                stg = stgpool.tile([1, TCH], F32R, tag="stg")
                nc.sync.dma_start(stg[:], recip_pp[r:r + 1, :])
                bc = spool.tile([P, 2 * TCH], F32, tag="s")
                nc.tensor.matmul(bc[0:DK, 0:TCH], ones_sb[0:1, 0:DK], stg[:],
                                 start=True, stop=True)
                sl = slice(qc * TCH, (qc + 1) * TCH)
                nc.vector.tensor_mul(oraw[h * DK:(h + 1) * DK, sl],
                                     oraw[h * DK:(h + 1) * DK, sl],
                                     bc[0:DK, 0:TCH])

        # ================= phase O: output projection for batch b ==========
        for m in range(KC):
            for qc in range(NQC):
                ps_y = ypool.tile([P, TCH], F32, tag="y")
                nc.tensor.matmul(ps_y[:], wo_sb[:, m * P:(m + 1) * P],
                                 oraw[:, qc * TCH:(qc + 1) * TCH],
                                 start=True, stop=True)
                ys = ystpool.tile([P, TCH], F32, tag="yst")
                nc.vector.tensor_scalar_add(ys[:], ps_y[:], bo_sb[:, m:m + 1])
                nc.sync.dma_start(
                    yt[m * P:(m + 1) * P, t0 + qc * TCH:t0 + (qc + 1) * TCH],
                    ys[:])

    for p in reversed(ctxs):
        p.__exit__(None, None, None)


_CACHED = {}


def _get_nc(repeat=1):
    if repeat not in _CACHED:
        _CACHED[repeat] = _build_nc(repeat=repeat)[0]
    return _CACHED[repeat]


def _make_in_maps(x, wq, bq, wk, bk, wv, bv, wo, bo):
    x = np.asarray(x, np.float32)
    wq, bq = np.asarray(wq, np.float32), np.asarray(bq, np.float32)
    wk, bk = np.asarray(wk, np.float32), np.asarray(bk, np.float32)
    wv, bv = np.asarray(wv, np.float32), np.asarray(bv, np.float32)
    wo, bo = np.asarray(wo, np.float32), np.asarray(bo, np.float32)
    xT = np.ascontiguousarray(x.reshape(T, D).T)
    maps = []
    for c in range(NCORES):
        sl = slice(c * P, (c + 1) * P)
        maps.append({
            "xt": xT,
            "wqm": np.ascontiguousarray(wq[:, sl]),
            "wqb": np.ascontiguousarray(bq[sl])[None, :],
            "wkm": np.ascontiguousarray(wk[:, sl]),
            "wkb": np.ascontiguousarray(bk[sl])[None, :],
            "wvm": np.ascontiguousarray(wv[:, sl]),
            "wvb": np.ascontiguousarray(bv[sl])[None, :],
            "wo": np.ascontiguousarray(wo[sl, :]),
            "bo": (bo if c == 0 else np.zeros_like(bo)).reshape(KC, P).copy(),
        })
    return maps


def kernel(x, wq, bq, wk, bk, wv, bv, wo, bo):
    nc = _get_nc()
    in_maps = _make_in_maps(x, wq, bq, wk, bk, wv, bv, wo, bo)
    res = run_bass_kernel_spmd(nc, in_maps, core_ids=list(range(NCORES)),
                               trace=False)
    yT = res.results[0]["yt"].copy()
    for c in range(1, NCORES):
        yT += res.results[c]["yt"]
    return np.ascontiguousarray(yT.T).reshape(B, S, D)



# revision 5
# speedup vs baseline: 1.0114x; 1.0114x over previous
"""BART attention (B=4, S=2048, D=1024, H=16) on 8 Trainium2 NeuronCores.

Sharding: tensor-parallel across heads.  Core c owns heads {2c, 2c+1}, i.e.
projection output dims [128c, 128c+128) of wq/wk/wv and rows [128c, 128c+128)
of wo.  Each core computes its two heads' attention over the full batch and a
partial output projection; the host sums the 8 partial outputs and adds bo.

v2: fully software-pipelined schedule.  Work is emitted in 16 "groups", one
per (batch, q-chunk).  Each group's PE stream interleaves, at k-tile
granularity:
  - scores + exp + attn@v for (b, qc)          (ACT-paced, ~1us/ktile)
  - softmax-normalization + out-projection of the previous group
  - the projections of batch b+1's chunk qc    (PE/DMA work, no ACT)
so the Scalar engine (exp) and Tensor engine never drain between phases.

Device layout per core (all matmuls in float32r: full PE speed, ~1e-4 rel):
  qT, kT  [128 head-dims, 8192 tokens]   (transposed projections)
  v_comb  [tokens, 130] = [vA(64) | 1 | vB(64) | 1]  (ones col -> softmax sums)
  scoresT [k-tok, q-tok] per (batch, head): softmax denom = extra out row of
  the ones-augmented attn@v matmul; exp on ScalarE with fused 1/8 scale; the
  1/sum normalization is applied after attn@v (flash-attention style).
  PSUM: scores 2x[128,1024] (4 banks) | attn-out 2x[65,512] (2) |
        proj accumulator [128,512] (1) | outproj/transpose [128,512] (1).
q/k/v biases are folded into the PSUM->SBUF drains (per-partition scalars on
DVE); bo is added on the host after the partial sum.
"""
import math

import numpy as np

import concourse.bass as bass
import concourse.mybir as mybir
import concourse.tile as tile
from concourse.bass_utils import run_bass_kernel_spmd
from concourse.masks import make_identity
from concourse.vector_clock import ScopedClock

F32 = mybir.dt.float32
F32R = mybir.dt.float32r
EXPF = mybir.ActivationFunctionType.Exp

B, S, D = 4, 2048, 1024
T = B * S                      # 8192 tokens
NCORES = 8
P = 128                        # partitions / head-dims per core
DK = 64                        # head dim
KC = D // P                    # 8 contraction chunks for projections
TCH = 512                      # token chunk (projection N / q-chunk)
NQC = S // TCH                 # 4 q-chunks (= proj chunks) per batch
NKT = S // P                   # 16 k-tiles per batch
VW = 2 * DK + 2                # 130: [vA | 1 | vB | 1]

# ---------------------------------------------------------------------------
# walrus in this toolchain encodes at most ONE sync wait per instruction
# (two on EventSemaphore).  Tile emits more.  Legalize by carrying excess
# waits on same-engine NOPs inserted right before the instruction (engines
# execute in order, so this is equivalent), and by splitting the kernel-tail
# drain's global-clock waits across a chain of drains.
# ---------------------------------------------------------------------------
_split_counter = [0]


def _legalize_waits(nc):
    inserted = 0
    for fn in nc.m.functions:
        for bb in fn.blocks:
            new_insts = []
            changed = False
            for inst in bb.instructions:
                si = inst.sync_info
                waits = list(si.on_wait) if si is not None and si.on_wait else []
                cap = 2 if inst.opcode == "EventSemaphore" else 1
                if len(waits) > cap:
                    excess, keep = waits[:-cap], waits[-cap:]
                    for w in excess:
                        _split_counter[0] += 1
                        nop = mybir.InstNoOp(
                            name=f"I-waitsplit-{_split_counter[0]}", ins=[], outs=[]
                        )
                        nop.engine = inst.engine
                        nop.sync_info = mybir.SyncInfo(on_wait=[w], on_update=[])
                        new_insts.append(nop)
                        inserted += 1
                    si.on_wait = keep
                    changed = True
                new_insts.append(inst)
            if changed:
                bb.instructions.clear()
                for i in new_insts:
                    bb.instructions.append(i)
    return inserted


class _TC(tile.TileContext):
    def _drain_and_barrier(self, tick_clock, wait_clock):
        drain_inst = self.nc.sync.drain()
        wait_clock.add_sem_waits(
            drain_inst.ins, ScopedClock({None: tick_clock.global_clock})
        )
        si = drain_inst.ins.sync_info
        waits = list(si.on_wait or []) if si is not None else []
        if len(waits) > 1:
            si.on_wait = [waits[0]]
            for w in waits[1:]:
                d = self.nc.sync.drain()
                dsi = d.ins.sync_info
                if dsi is None:
                    d.ins.sync_info = mybir.SyncInfo(on_wait=[w], on_update=[])
                else:
                    dsi.on_wait = [w]
        self.nc.all_engine_barrier()
        assert self.sems is not None
        popped = self.nc._tile_sem_poison_stack.pop()
        assert popped is self._sem_poison
        self.nc.clear_and_free_semaphores(list(self.sems.allocated().values()))
        self.nc.all_engine_barrier()


# ---------------------------------------------------------------------------
# device program (identical on all 8 cores; only input data differs)
# ---------------------------------------------------------------------------
def _build_nc(repeat=1):
    nc = bass.Bass("TRN2", target_bir_lowering=False, debug=False,
                   num_devices=NCORES)
    xt = nc.dram_tensor("xt", [D, T], F32R, kind="ExternalInput").ap()
    wqm = nc.dram_tensor("wqm", [D, P], F32R, kind="ExternalInput").ap()
    wqb = nc.dram_tensor("wqb", [P, 1], F32, kind="ExternalInput").ap()
    wkm = nc.dram_tensor("wkm", [D, P], F32R, kind="ExternalInput").ap()
    wkb = nc.dram_tensor("wkb", [P, 1], F32, kind="ExternalInput").ap()
    wvm = nc.dram_tensor("wvm", [D, P], F32R, kind="ExternalInput").ap()
    wvb = nc.dram_tensor("wvb", [P, 1], F32, kind="ExternalInput").ap()
    wot = nc.dram_tensor("wo", [P, D], F32R, kind="ExternalInput").ap()
    yt = nc.dram_tensor("yt", [D, T], F32, kind="ExternalOutput").ap()

    with _TC(nc) as tc, nc.allow_low_precision(
            reason="float32r is 32-bit; PE rounds internally"):
        _emit(nc, tc, xt, wqm, wqb, wkm, wkb, wvm, wvb, wot, yt,
              repeat=repeat)
    n = _legalize_waits(nc)
    return nc, n


def _emit(nc, tc, xt, wqm, wqb, wkm, wkb, wvm, wvb, wot, yt, repeat=1):
    ctxs = []

    def pool(name, bufs, space="SBUF"):
        p = tc.tile_pool(name=name, bufs=bufs, space=space)
        ctxs.append(p)
        return p.__enter__()

    wpool = pool("w", 1)
    persist = pool("persist", 1)
    xpool = pool("x", 2)
    epool = pool("e", 3)
    scrpool = pool("scr", 2)
    orawpool = pool("oraw", 2)
    yspool = pool("ys", 3)
    stgpool = pool("stg", 4)
    psum = pool("ps", 1, space="PSUM")

    def ps_tile(shape, tag, bufs, name):
        return psum.tile(shape, F32, tag=tag, bufs=bufs, name=name)

    # ---- constants / weights (loaded once) ----
    wq_sb = wpool.tile([P, KC, P], F32R)
    wk_sb = wpool.tile([P, KC, P], F32R)
    wv_sb = wpool.tile([P, KC, P], F32R)
    nc.sync.dma_start(wq_sb[:], wqm.rearrange("(k p) d -> p k d", p=P))
    nc.sync.dma_start(wk_sb[:], wkm.rearrange("(k p) d -> p k d", p=P))
    nc.sync.dma_start(wv_sb[:], wvm.rearrange("(k p) d -> p k d", p=P))
    wqb_sb = wpool.tile([P, 1], F32)
    wkb_sb = wpool.tile([P, 1], F32)
    wvb_sb = wpool.tile([P, 1], F32)
    nc.sync.dma_start(wqb_sb[:], wqb[:, :])
    nc.sync.dma_start(wkb_sb[:], wkb[:, :])
    nc.sync.dma_start(wvb_sb[:], wvb[:, :])
    wo_sb = wpool.tile([P, D], F32R)
    nc.sync.dma_start(wo_sb[:], wot[:, :])
    # memset can't write float32r; memset f32 then DVE-copy (which rounds)
    ones_f32 = wpool.tile([P, TCH], F32)
    nc.vector.memset(ones_f32[:], 1.0)
    ones_sb = wpool.tile([1, TCH], F32R)
    nc.vector.tensor_copy(ones_sb[:], ones_f32[0:1, :])
    ident = wpool.tile([P, P], F32)
    make_identity(nc, ident[:])
    ident_r = wpool.tile([P, P], F32R)
    nc.vector.tensor_copy(ident_r[:], ident[:])

    # ---- persistent activations ----
    qT = persist.tile([P, T], F32R)
    kT = persist.tile([P, T], F32R)
    v_comb = persist.tile([P, T // P, VW], F32R)    # [tok%128, tok-tile, 130]
    nc.vector.tensor_copy(
        v_comb[:, :, DK:DK + 1],
        ones_f32[:, 0:1].broadcast_to([P, T // P, 1]))
    nc.vector.tensor_copy(
        v_comb[:, :, VW - 1:VW],
        ones_f32[:, 0:1].broadcast_to([P, T // P, 1]))

    NB = B * repeat

    # ---------------- projection of batch bg1 (mod B), chunk c -------------
    def make_proj_units(bg1, c):
        b1 = bg1 % B
        c0 = b1 * S + c * TCH
        h = {}
        units = []

        def u_load():
            x_ch = xpool.tile([P, KC, TCH], F32R, tag="x", name="x_ch")
            nc.sync.dma_start(
                x_ch[:], xt[:, c0:c0 + TCH].rearrange("(k p) n -> p k n", p=P))
            h["x"] = x_ch
        units.append(u_load)

        def mk_pass(w_sb, drain_fn, key):
            def u_start():
                h[key] = ps_tile([P, TCH], "p", 1, f"prj_{key}")
                nc.tensor.matmul(h[key][:], w_sb[:, 0, :], h["x"][:, 0, :],
                                 start=True, stop=False)
                nc.tensor.matmul(h[key][:], w_sb[:, 1, :], h["x"][:, 1, :],
                                 start=False, stop=False)
            units.append(u_start)
            for k0 in (2, 4, 6):
                def u_mid(k0=k0):
                    nc.tensor.matmul(h[key][:], w_sb[:, k0, :],
                                     h["x"][:, k0, :], start=False, stop=False)
                    nc.tensor.matmul(h[key][:], w_sb[:, k0 + 1, :],
                                     h["x"][:, k0 + 1, :],
                                     start=False, stop=(k0 == 6))
                units.append(u_mid)
            units.append(drain_fn)

        def drain_q():
            nc.vector.tensor_scalar_add(qT[:, c0:c0 + TCH], h["q"][:],
                                        wqb_sb[:, 0:1])
        def drain_k():
            nc.vector.tensor_scalar_add(kT[:, c0:c0 + TCH], h["k"][:],
                                        wkb_sb[:, 0:1])
        def drain_v():
            v_scr = scrpool.tile([P, TCH], F32R, tag="vscr", name="v_scr")
            nc.vector.tensor_scalar_add(v_scr[:], h["v"][:], wvb_sb[:, 0:1])
            h["vs"] = v_scr

        mk_pass(wq_sb, drain_q, "q")
        mk_pass(wk_sb, drain_k, "k")
        mk_pass(wv_sb, drain_v, "v")

        for tt in range(TCH // P):
            def u_tr(tt=tt):
                vt = (c0 // P) + tt
                tr = psum.tile([P, TCH], F32R, tag="y", bufs=1, name="tr")
                nc.tensor.transpose(tr[:, 0:P],
                                    h["vs"][:, tt * P:(tt + 1) * P], ident_r[:])
                nc.vector.tensor_copy(v_comb[:, vt, 0:DK], tr[:, 0:DK])
                nc.vector.tensor_copy(v_comb[:, vt, DK + 1:2 * DK + 1],
                                      tr[:, DK:2 * DK])
            units.append(u_tr)
        return units

    # -------- normalization + out-projection of a finished group ----------
    def make_norm_outproj_units(prev, noproj):
        b, qc, ps_oA, ps_oB = prev
        t0 = b * S
        q0 = t0 + qc * TCH
        h = {}
        units = []

        def u_recip():
            stgA = stgpool.tile([1, TCH], F32R, tag="stg", name="stgA")
            stgB = stgpool.tile([1, TCH], F32R, tag="stg", name="stgB")
            nc.vector.reciprocal(stgA[:], ps_oA[DK:DK + 1, :])
            nc.vector.reciprocal(stgB[:], ps_oB[DK:DK + 1, :])
            h["rA"], h["rB"] = stgA, stgB
        units.append(u_recip)

        def u_bc():
            bc = ps_tile([P, 2 * TCH], "s", 2, "bc")
            nc.tensor.matmul(bc[0:DK, 0:TCH], ones_sb[0:1, 0:DK], h["rA"][:],
                             start=True, stop=True)
            nc.tensor.matmul(bc[0:DK, TCH:2 * TCH], ones_sb[0:1, 0:DK],
                             h["rB"][:], start=True, stop=True)
            h["bc"] = bc
        units.append(u_bc)

        def u_cpA():
            oraw = orawpool.tile([P, TCH], F32R, tag="oraw", name="oraw")
            nc.vector.tensor_copy(oraw[0:DK, :], ps_oA[0:DK, :])
            h["o"] = oraw
        units.append(u_cpA)

        def u_cpB():
            nc.vector.tensor_copy(h["o"][DK:P, :], ps_oB[0:DK, :])
        units.append(u_cpB)

        def u_mulA():
            nc.vector.tensor_mul(h["o"][0:DK, :], h["o"][0:DK, :],
                                 h["bc"][0:DK, 0:TCH])
        units.append(u_mulA)

        def u_mulB():
            nc.vector.tensor_mul(h["o"][DK:P, :], h["o"][DK:P, :],
                                 h["bc"][0:DK, TCH:2 * TCH])
        units.append(u_mulB)

        for m in range(KC):
            def u_out(m=m):
                tag = ("y", "p")[m % 2] if noproj else "y"
                ps_y = ps_tile([P, TCH], tag, 1, "ps_y")
                nc.tensor.matmul(ps_y[:], wo_sb[:, m * P:(m + 1) * P],
                                 h["o"][:], start=True, stop=True)
                ys = yspool.tile([P, TCH], F32, tag="yst", name="ys")
                nc.vector.tensor_copy(ys[:], ps_y[:])
                nc.sync.dma_start(yt[m * P:(m + 1) * P, q0:q0 + TCH], ys[:])
            units.append(u_out)
        return units

    # ---------------- main pipelined schedule ------------------------------
    # prologue: projections of batch 0 (all 4 chunks, un-interleaved)
    for c in range(NQC):
        for u in make_proj_units(0, c):
            u()

    prev = None
    for bg in range(NB):
        b = bg % B
        t0 = b * S
        for qc in range(NQC):
            q0 = t0 + qc * TCH
            extras = []
            if prev is not None:
                extras += make_norm_outproj_units(prev, noproj=(bg + 1 >= NB))
            if bg + 1 < NB:
                extras += make_proj_units(bg + 1, qc)

            ps_oA = ps_tile([DK + 1, TCH], "o", 2, "ps_oA")
            ps_oB = ps_tile([DK + 1, TCH], "o", 2, "ps_oB")

            def attnv(kc, e_t):
                vt = (t0 // P) + kc
                nc.tensor.matmul(ps_oA[:], v_comb[:, vt, 0:DK + 1],
                                 e_t[:, 0:TCH],
                                 start=(kc == 0), stop=(kc == NKT - 1))
                nc.tensor.matmul(ps_oB[:], v_comb[:, vt, DK + 1:VW],
                                 e_t[:, TCH:2 * TCH],
                                 start=(kc == 0), stop=(kc == NKT - 1))

            pending = None
            for kc in range(NKT):
                kt0 = t0 + kc * P
                s_t = ps_tile([P, 2 * TCH], "s", 2, "s_t")
                nc.tensor.matmul(s_t[:, 0:TCH], kT[0:DK, kt0:kt0 + P],
                                 qT[0:DK, q0:q0 + TCH], start=True, stop=True)
                nc.tensor.matmul(s_t[:, TCH:2 * TCH], kT[DK:P, kt0:kt0 + P],
                                 qT[DK:P, q0:q0 + TCH], start=True, stop=True)
                e_t = epool.tile([P, 2 * TCH], F32R, tag="e", name="e_t")
                nc.scalar.activation(e_t[:], s_t[:], EXPF, scale=0.125)
                if pending is not None:
                    attnv(*pending)
                pending = (kc, e_t)
                n_pop = math.ceil(len(extras) / (NKT - kc))
                for _ in range(min(n_pop, len(extras))):
                    extras.pop(0)()
            attnv(*pending)
            for u in extras:
                u()
            prev = (b, qc, ps_oA, ps_oB)

    for u in make_norm_outproj_units(prev, noproj=True):
        u()

    for p in reversed(ctxs):
        p.__exit__(None, None, None)


_CACHED = {}


def _get_nc(repeat=1):
    if repeat not in _CACHED:
        _CACHED[repeat] = _build_nc(repeat=repeat)[0]
    return _CACHED[repeat]


def _make_in_maps(x, wq, bq, wk, bk, wv, bv, wo, bo):
    x = np.asarray(x, np.float32)
    wq, bq = np.asarray(wq, np.float32), np.asarray(bq, np.float32)
    wk, bk = np.asarray(wk, np.float32), np.asarray(bk, np.float32)
    wv, bv = np.asarray(wv, np.float32), np.asarray(bv, np.float32)
    wo = np.asarray(wo, np.float32)
    xT = np.ascontiguousarray(x.reshape(T, D).T)
    maps = []
    for c in range(NCORES):
        sl = slice(c * P, (c + 1) * P)
        maps.append({
            "xt": xT,
            "wqm": np.ascontiguousarray(wq[:, sl]),
            "wqb": np.ascontiguousarray(bq[sl])[:, None],
            "wkm": np.ascontiguousarray(wk[:, sl]),
            "wkb": np.ascontiguousarray(bk[sl])[:, None],
            "wvm": np.ascontiguousarray(wv[:, sl]),
            "wvb": np.ascontiguousarray(bv[sl])[:, None],
            "wo": np.ascontiguousarray(wo[sl, :]),
        })
    return maps


def kernel(x, wq, bq, wk, bk, wv, bv, wo, bo):
    nc = _get_nc()
    in_maps = _make_in_maps(x, wq, bq, wk, bk, wv, bv, wo, bo)
    res = run_bass_kernel_spmd(nc, in_maps, core_ids=list(range(NCORES)),
                               trace=False)
    yT = res.results[0]["yt"].copy()
    for c in range(1, NCORES):
        yT += res.results[c]["yt"]
    y = np.ascontiguousarray(yT.T).reshape(B, S, D)
    return y + np.asarray(bo, np.float32)[None, None, :]


# revision 9
# speedup vs baseline: 1.4391x; 1.4228x over previous
"""BART attention (B=4, S=2048, D=1024, H=16) on 8 Trainium2 NeuronCores.

Sharding: tensor-parallel across heads.  Core c owns heads {2c, 2c+1}, i.e.
projection output dims [128c, 128c+128) of wq/wk/wv and rows [128c, 128c+128)
of wo.  Each core computes its two heads' attention over the full batch and a
partial output projection; the host sums the 8 partial outputs and adds bo.

v2: fully software-pipelined schedule.  Work is emitted in 16 "groups", one
per (batch, q-chunk).  Each group's PE stream interleaves, at k-tile
granularity:
  - scores + exp + attn@v for (b, qc)          (ACT-paced, ~1us/ktile)
  - softmax-normalization + out-projection of the previous group
  - the projections of batch b+1's chunk qc    (PE/DMA work, no ACT)
so the Scalar engine (exp) and Tensor engine never drain between phases.

Device layout per core (all matmuls in float32r: full PE speed, ~1e-4 rel):
  qT, kT  [128 head-dims, 8192 tokens]   (transposed projections)
  v_comb  [tokens, 130] = [vA(64) | 1 | vB(64) | 1]  (ones col -> softmax sums)
  scoresT [k-tok, q-tok] per (batch, head): softmax denom = extra out row of
  the ones-augmented attn@v matmul; exp on ScalarE with fused 1/8 scale; the
  1/sum normalization is applied after attn@v (flash-attention style).
  PSUM: scores 2x[128,1024] (4 banks) | attn-out 2x[65,512] (2) |
        proj accumulator [128,512] (1) | outproj/transpose [128,512] (1).
q/k/v biases are folded into the PSUM->SBUF drains (per-partition scalars on
DVE); bo is added on the host after the partial sum.
"""
import math

import numpy as np

import concourse.bass as bass
import concourse.mybir as mybir
import concourse.tile as tile
from concourse.bass_utils import run_bass_kernel_spmd
from concourse.masks import make_identity
from concourse.vector_clock import ScopedClock

F32 = mybir.dt.float32
F32R = mybir.dt.float32r
BF16 = mybir.dt.bfloat16
EXPF = mybir.ActivationFunctionType.Exp

B, S, D = 4, 2048, 1024
T = B * S                      # 8192 tokens
NCORES = 8
P = 128                        # partitions / head-dims per core
DK = 64                        # head dim
KC = D // P                    # 8 contraction chunks for projections
TCH = 512                      # token chunk (projection N / q-chunk)
NQC = S // TCH                 # 4 q-chunks (= proj chunks) per batch
NKT = S // P                   # 16 k-tiles per batch
VW = 2 * DK + 2                # 130: [vA | 1 | vB | 1]

# ---------------------------------------------------------------------------
# walrus in this toolchain encodes at most ONE sync wait per instruction
# (two on EventSemaphore).  Tile emits more.  Legalize by carrying excess
# waits on same-engine NOPs inserted right before the instruction (engines
# execute in order, so this is equivalent), and by splitting the kernel-tail
# drain's global-clock waits across a chain of drains.
# ---------------------------------------------------------------------------
_split_counter = [0]


def _legalize_waits(nc):
    inserted = 0
    for fn in nc.m.functions:
        for bb in fn.blocks:
            new_insts = []
            changed = False
            for inst in bb.instructions:
                si = inst.sync_info
                waits = list(si.on_wait) if si is not None and si.on_wait else []
                cap = 2 if inst.opcode == "EventSemaphore" else 1
                if len(waits) > cap:
                    excess, keep = waits[:-cap], waits[-cap:]
                    for w in excess:
                        _split_counter[0] += 1
                        nop = mybir.InstNoOp(
                            name=f"I-waitsplit-{_split_counter[0]}", ins=[], outs=[]
                        )
                        nop.engine = inst.engine
                        nop.sync_info = mybir.SyncInfo(on_wait=[w], on_update=[])
                        new_insts.append(nop)
                        inserted += 1
                    si.on_wait = keep
                    changed = True
                new_insts.append(inst)
            if changed:
                bb.instructions.clear()
                for i in new_insts:
                    bb.instructions.append(i)
    return inserted


class _TC(tile.TileContext):
    def _drain_and_barrier(self, tick_clock, wait_clock):
        drain_inst = self.nc.sync.drain()
        wait_clock.add_sem_waits(
            drain_inst.ins, ScopedClock({None: tick_clock.global_clock})
        )
        si = drain_inst.ins.sync_info
        waits = list(si.on_wait or []) if si is not None else []
        if len(waits) > 1:
            si.on_wait = [waits[0]]
            for w in waits[1:]:
                d = self.nc.sync.drain()
                dsi = d.ins.sync_info
                if dsi is None:
                    d.ins.sync_info = mybir.SyncInfo(on_wait=[w], on_update=[])
                else:
                    dsi.on_wait = [w]
        self.nc.all_engine_barrier()
        assert self.sems is not None
        popped = self.nc._tile_sem_poison_stack.pop()
        assert popped is self._sem_poison
        self.nc.clear_and_free_semaphores(list(self.sems.allocated().values()))
        self.nc.all_engine_barrier()


# ---------------------------------------------------------------------------
# device program (identical on all 8 cores; only input data differs)
# ---------------------------------------------------------------------------
def _build_nc(repeat=1):
    nc = bass.Bass("TRN2", target_bir_lowering=False, debug=False,
                   num_devices=NCORES)
    xt = nc.dram_tensor("xt", [D, T], BF16, kind="ExternalInput").ap()
    wqm = nc.dram_tensor("wqm", [D, P], BF16, kind="ExternalInput").ap()
    wqb = nc.dram_tensor("wqb", [P, 1], F32, kind="ExternalInput").ap()
    wkm = nc.dram_tensor("wkm", [D, P], BF16, kind="ExternalInput").ap()
    wkb = nc.dram_tensor("wkb", [P, 1], F32, kind="ExternalInput").ap()
    wvm = nc.dram_tensor("wvm", [D, P], BF16, kind="ExternalInput").ap()
    wvb = nc.dram_tensor("wvb", [P, 1], F32, kind="ExternalInput").ap()
    wot = nc.dram_tensor("wo", [P, D], BF16, kind="ExternalInput").ap()
    yt = nc.dram_tensor("yt", [D, T], BF16, kind="ExternalOutput").ap()

    with _TC(nc) as tc, nc.allow_low_precision(
            reason="float32r is 32-bit; PE rounds internally"):
        _emit(nc, tc, xt, wqm, wqb, wkm, wkb, wvm, wvb, wot, yt,
              repeat=repeat)
    n = _legalize_waits(nc)
    return nc, n


def _emit(nc, tc, xt, wqm, wqb, wkm, wkb, wvm, wvb, wot, yt, repeat=1):
    ctxs = []

    def pool(name, bufs, space="SBUF"):
        p = tc.tile_pool(name=name, bufs=bufs, space=space)
        ctxs.append(p)
        return p.__enter__()

    wpool = pool("w", 1)
    persist = pool("persist", 1)
    xpool = pool("x", 2)
    epool = pool("e", 3)
    scrpool = pool("scr", 2)
    orawpool = pool("oraw", 2)
    yspool = pool("ys", 3)
    stgpool = pool("stg", 4)
    psum = pool("ps", 1, space="PSUM")

    def ps_tile(shape, tag, bufs, name):
        return psum.tile(shape, F32, tag=tag, bufs=bufs, name=name)

    # ---- constants / weights (loaded once) ----
    wq_sb = wpool.tile([P, KC, P], BF16)
    wk_sb = wpool.tile([P, KC, P], BF16)
    wv_sb = wpool.tile([P, KC, P], BF16)
    nc.sync.dma_start(wq_sb[:], wqm.rearrange("(k p) d -> p k d", p=P))
    nc.sync.dma_start(wk_sb[:], wkm.rearrange("(k p) d -> p k d", p=P))
    nc.sync.dma_start(wv_sb[:], wvm.rearrange("(k p) d -> p k d", p=P))
    wqb_sb = wpool.tile([P, 1], F32)
    wkb_sb = wpool.tile([P, 1], F32)
    wvb_sb = wpool.tile([P, 1], F32)
    nc.sync.dma_start(wqb_sb[:], wqb[:, :])
    nc.sync.dma_start(wkb_sb[:], wkb[:, :])
    nc.sync.dma_start(wvb_sb[:], wvb[:, :])
    wo_sb = wpool.tile([P, D], BF16)
    nc.sync.dma_start(wo_sb[:], wot[:, :])
    # memset can't write float32r; memset f32 then DVE-copy (which rounds)
    ones_f32 = wpool.tile([P, TCH], F32)
    nc.vector.memset(ones_f32[:], 1.0)
    ones_sb = wpool.tile([1, TCH], BF16)
    nc.vector.tensor_copy(ones_sb[:], ones_f32[0:1, :])
    ident = wpool.tile([P, P], F32)
    make_identity(nc, ident[:])
    ident_r = wpool.tile([P, P], BF16)
    nc.vector.tensor_copy(ident_r[:], ident[:])

    # ---- persistent activations ----
    qT = persist.tile([P, T], BF16)
    kT = persist.tile([P, T], BF16)
    v_comb = persist.tile([P, T // P, VW], BF16)    # [tok%128, tok-tile, 130]
    nc.vector.tensor_copy(
        v_comb[:, :, DK:DK + 1],
        ones_f32[:, 0:1].broadcast_to([P, T // P, 1]))
    nc.vector.tensor_copy(
        v_comb[:, :, VW - 1:VW],
        ones_f32[:, 0:1].broadcast_to([P, T // P, 1]))

    NB = B * repeat

    # ---------------- projection of batch bg1 (mod B), chunk c -------------
    def make_proj_units(bg1, c):
        b1 = bg1 % B
        c0 = b1 * S + c * TCH
        h = {}
        units = []

        def u_load():
            x_ch = xpool.tile([P, KC, TCH], BF16, tag="x", name="x_ch")
            nc.sync.dma_start(
                x_ch[:], xt[:, c0:c0 + TCH].rearrange("(k p) n -> p k n", p=P))
            h["x"] = x_ch
        units.append(u_load)

        def mk_pass(w_sb, drain_fn, key):
            def u_start():
                h[key] = ps_tile([P, TCH], "p", 1, f"prj_{key}")
                nc.tensor.matmul(h[key][:], w_sb[:, 0, :], h["x"][:, 0, :],
                                 start=True, stop=False)
                nc.tensor.matmul(h[key][:], w_sb[:, 1, :], h["x"][:, 1, :],
                                 start=False, stop=False)
            units.append(u_start)
            for k0 in (2, 4, 6):
                def u_mid(k0=k0):
                    nc.tensor.matmul(h[key][:], w_sb[:, k0, :],
                                     h["x"][:, k0, :], start=False, stop=False)
                    nc.tensor.matmul(h[key][:], w_sb[:, k0 + 1, :],
                                     h["x"][:, k0 + 1, :],
                                     start=False, stop=(k0 == 6))
                units.append(u_mid)
            units.append(drain_fn)

        def drain_q():
            nc.vector.tensor_scalar_add(qT[:, c0:c0 + TCH], h["q"][:],
                                        wqb_sb[:, 0:1])
        def drain_k():
            nc.vector.tensor_scalar_add(kT[:, c0:c0 + TCH], h["k"][:],
                                        wkb_sb[:, 0:1])
        def drain_v():
            v_scr = scrpool.tile([P, TCH], BF16, tag="vscr", name="v_scr")
            nc.vector.tensor_scalar_add(v_scr[:], h["v"][:], wvb_sb[:, 0:1])
            h["vs"] = v_scr

        mk_pass(wq_sb, drain_q, "q")
        mk_pass(wk_sb, drain_k, "k")
        mk_pass(wv_sb, drain_v, "v")

        for tt in range(TCH // P):
            def u_tr(tt=tt):
                vt = (c0 // P) + tt
                tr = psum.tile([P, TCH], BF16, tag="y", bufs=1, name="tr")
                nc.tensor.transpose(tr[:, 0:P],
                                    h["vs"][:, tt * P:(tt + 1) * P], ident_r[:])
                nc.vector.tensor_copy(v_comb[:, vt, 0:DK], tr[:, 0:DK])
                nc.vector.tensor_copy(v_comb[:, vt, DK + 1:2 * DK + 1],
                                      tr[:, DK:2 * DK])
            units.append(u_tr)
        return units

    # -------- normalization + out-projection of a finished group ----------
    def make_norm_outproj_units(prev, noproj):
        b, qc, ps_oA, ps_oB = prev
        t0 = b * S
        q0 = t0 + qc * TCH
        h = {}
        units = []

        def u_recip():
            stgA = stgpool.tile([1, TCH], BF16, tag="stg", name="stgA")
            stgB = stgpool.tile([1, TCH], BF16, tag="stg", name="stgB")
            nc.vector.reciprocal(stgA[:], ps_oA[DK:DK + 1, :])
            nc.vector.reciprocal(stgB[:], ps_oB[DK:DK + 1, :])
            h["rA"], h["rB"] = stgA, stgB
        units.append(u_recip)

        def u_bc():
            bcA = ps_tile([DK, TCH], "p", 1, "bcA")
            bcB = ps_tile([DK, TCH], "y", 1, "bcB")
            nc.tensor.matmul(bcA[:, :], ones_sb[0:1, 0:DK], h["rA"][:],
                             start=True, stop=True)
            nc.tensor.matmul(bcB[:, :], ones_sb[0:1, 0:DK], h["rB"][:],
                             start=True, stop=True)
            h["bcA"], h["bcB"] = bcA, bcB
        units.append(u_bc)

        def u_cpA():
            oraw = orawpool.tile([P, TCH], BF16, tag="oraw", name="oraw")
            nc.vector.tensor_copy(oraw[0:DK, :], ps_oA[0:DK, :])
            h["o"] = oraw
        units.append(u_cpA)

        def u_cpB():
            nc.vector.tensor_copy(h["o"][DK:P, :], ps_oB[0:DK, :])
        units.append(u_cpB)

        def u_mulA():
            nc.vector.tensor_mul(h["o"][0:DK, :], h["o"][0:DK, :],
                                 h["bcA"][:, :])
        units.append(u_mulA)

        def u_mulB():
            nc.vector.tensor_mul(h["o"][DK:P, :], h["o"][DK:P, :],
                                 h["bcB"][:, :])
        units.append(u_mulB)

        for m in range(KC):
            def u_out(m=m):
                tag = ("y", "p")[m % 2] if noproj else "y"
                ps_y = ps_tile([P, TCH], tag, 1, "ps_y")
                nc.tensor.matmul(ps_y[:], wo_sb[:, m * P:(m + 1) * P],
                                 h["o"][:], start=True, stop=True)
                ys = yspool.tile([P, TCH], BF16, tag="yst", name="ys")
                nc.vector.tensor_copy(ys[:], ps_y[:])
                nc.sync.dma_start(yt[m * P:(m + 1) * P, q0:q0 + TCH], ys[:])
            units.append(u_out)
        return units

    # ---------------- main pipelined schedule ------------------------------
    # prologue: projections of batch 0 (all 4 chunks, un-interleaved)
    for c in range(NQC):
        for u in make_proj_units(0, c):
            u()

    prev = None
    for bg in range(NB):
        b = bg % B
        t0 = b * S
        for qc in range(NQC):
            q0 = t0 + qc * TCH
            extras = []
            if prev is not None:
                extras += make_norm_outproj_units(prev, noproj=(bg + 1 >= NB))
            if bg + 1 < NB:
                extras += make_proj_units(bg + 1, qc)

            ps_oA = ps_tile([DK + 1, TCH], "o", 2, "ps_oA")
            ps_oB = ps_tile([DK + 1, TCH], "o", 2, "ps_oB")

            def attnv(kc, e_t):
                vt = (t0 // P) + kc
                nc.tensor.matmul(ps_oA[:], v_comb[:, vt, 0:DK + 1],
                                 e_t[:, 0:TCH],
                                 start=(kc == 0), stop=(kc == NKT - 1))
                nc.tensor.matmul(ps_oB[:], v_comb[:, vt, DK + 1:VW],
                                 e_t[:, TCH:2 * TCH],
                                 start=(kc == 0), stop=(kc == NKT - 1))

            pending = None
            for kc in range(NKT):
                kt0 = t0 + kc * P
                s_t = ps_tile([P, 2 * TCH], "s", 2, "s_t")
                nc.tensor.matmul(s_t[:, 0:TCH], kT[0:DK, kt0:kt0 + P],
                                 qT[0:DK, q0:q0 + TCH], start=True, stop=True)
                nc.tensor.matmul(s_t[:, TCH:2 * TCH], kT[DK:P, kt0:kt0 + P],
                                 qT[DK:P, q0:q0 + TCH], start=True, stop=True)
                e_t = epool.tile([P, 2 * TCH], BF16, tag="e", name="e_t")
                nc.scalar.activation(e_t[:], s_t[:], EXPF, scale=0.125)
                if pending is not None:
                    attnv(*pending)
                pending = (kc, e_t)
                n_pop = math.ceil(len(extras) / (NKT - kc))
                for _ in range(min(n_pop, len(extras))):
                    extras.pop(0)()
            attnv(*pending)
            for u in extras:
                u()
            prev = (b, qc, ps_oA, ps_oB)

    for u in make_norm_outproj_units(prev, noproj=True):
        u()

    for p in reversed(ctxs):
        p.__exit__(None, None, None)


_CACHED = {}


def _get_nc(repeat=1):
    if repeat not in _CACHED:
        _CACHED[repeat] = _build_nc(repeat=repeat)[0]
    return _CACHED[repeat]


def _make_in_maps(x, wq, bq, wk, bk, wv, bv, wo, bo):
    from ml_dtypes import bfloat16
    x = np.asarray(x, np.float32)
    wq, bq = np.asarray(wq, bfloat16), np.asarray(bq, np.float32)
    wk, bk = np.asarray(wk, bfloat16), np.asarray(bk, np.float32)
    wv, bv = np.asarray(wv, bfloat16), np.asarray(bv, np.float32)
    wo = np.asarray(wo, bfloat16)
    xT = np.ascontiguousarray(x.reshape(T, D).T).astype(bfloat16)
    maps = []
    for c in range(NCORES):
        sl = slice(c * P, (c + 1) * P)
        maps.append({
            "xt": xT,
            "wqm": np.ascontiguousarray(wq[:, sl]),
            "wqb": np.ascontiguousarray(bq[sl])[:, None],
            "wkm": np.ascontiguousarray(wk[:, sl]),
            "wkb": np.ascontiguousarray(bk[sl])[:, None],
            "wvm": np.ascontiguousarray(wv[:, sl]),
            "wvb": np.ascontiguousarray(bv[sl])[:, None],
            "wo": np.ascontiguousarray(wo[sl, :]),
        })
    return maps


def kernel(x, wq, bq, wk, bk, wv, bv, wo, bo):
    nc = _get_nc()
    in_maps = _make_in_maps(x, wq, bq, wk, bk, wv, bv, wo, bo)
    res = run_bass_kernel_spmd(nc, in_maps, core_ids=list(range(NCORES)),
                               trace=False)
    yT = res.results[0]["yt"].astype(np.float32)
    for c in range(1, NCORES):
        yT += res.results[c]["yt"].astype(np.float32)
    y = np.ascontiguousarray(yT.T).reshape(B, S, D)
    return y + np.asarray(bo, np.float32)[None, None, :]


# revision 12
# speedup vs baseline: 1.5780x; 1.0965x over previous
"""BART attention (B=4, S=2048, D=1024, H=16) on 8 Trainium2 NeuronCores.

Sharding: tensor-parallel across heads.  Core c owns heads {2c, 2c+1}, i.e.
projection output dims [128c, 128c+128) of wq/wk/wv and rows [128c, 128c+128)
of wo.  Each core computes its two heads' attention over the full batch and a
partial output projection; the host sums the 8 partial outputs and adds bo.

v2: fully software-pipelined schedule.  Work is emitted in 16 "groups", one
per (batch, q-chunk).  Each group's PE stream interleaves, at k-tile
granularity:
  - scores + exp + attn@v for (b, qc)          (ACT-paced, ~1us/ktile)
  - softmax-normalization + out-projection of the previous group
  - the projections of batch b+1's chunk qc    (PE/DMA work, no ACT)
so the Scalar engine (exp) and Tensor engine never drain between phases.

Device layout per core (all matmuls in float32r: full PE speed, ~1e-4 rel):
  qT, kT  [128 head-dims, 8192 tokens]   (transposed projections)
  v_comb  [tokens, 130] = [vA(64) | 1 | vB(64) | 1]  (ones col -> softmax sums)
  scoresT [k-tok, q-tok] per (batch, head): softmax denom = extra out row of
  the ones-augmented attn@v matmul; exp on ScalarE with fused 1/8 scale; the
  1/sum normalization is applied after attn@v (flash-attention style).
  PSUM: scores 2x[128,1024] (4 banks) | attn-out 2x[65,512] (2) |
        proj accumulator [128,512] (1) | outproj/transpose [128,512] (1).
q/k/v biases are folded into the PSUM->SBUF drains (per-partition scalars on
DVE); bo is added on the host after the partial sum.
"""
import math

import numpy as np

import concourse.bass as bass
import concourse.mybir as mybir
import concourse.tile as tile
import os
from concourse.bass_utils import run_bass_kernel_spmd
from concourse.masks import make_identity
from concourse.vector_clock import ScopedClock

F32 = mybir.dt.float32
F32R = mybir.dt.float32r
BF16 = mybir.dt.bfloat16
EXPF = mybir.ActivationFunctionType.Exp

B, S, D = 4, 2048, 1024
T = B * S                      # 8192 tokens
NCORES = 8
P = 128                        # partitions / head-dims per core
DK = 64                        # head dim
KC = D // P                    # 8 contraction chunks for projections
TCH = 512                      # token chunk (projection N / q-chunk)
NQC = S // TCH                 # 4 q-chunks (= proj chunks) per batch
NKT = S // P                   # 16 k-tiles per batch
VW = 2 * DK + 2                # 130: [vA | 1 | vB | 1]
ATTN_ONLY = bool(int(os.environ.get('ATTN_ONLY', '0')))

# ---------------------------------------------------------------------------
# walrus in this toolchain encodes at most ONE sync wait per instruction
# (two on EventSemaphore).  Tile emits more.  Legalize by carrying excess
# waits on same-engine NOPs inserted right before the instruction (engines
# execute in order, so this is equivalent), and by splitting the kernel-tail
# drain's global-clock waits across a chain of drains.
# ---------------------------------------------------------------------------
_split_counter = [0]


def _legalize_waits(nc):
    inserted = 0
    for fn in nc.m.functions:
        for bb in fn.blocks:
            new_insts = []
            changed = False
            for inst in bb.instructions:
                si = inst.sync_info
                waits = list(si.on_wait) if si is not None and si.on_wait else []
                cap = 2 if inst.opcode == "EventSemaphore" else 1
                if len(waits) > cap:
                    excess, keep = waits[:-cap], waits[-cap:]
                    for w in excess:
                        _split_counter[0] += 1
                        nop = mybir.InstNoOp(
                            name=f"I-waitsplit-{_split_counter[0]}", ins=[], outs=[]
                        )
                        nop.engine = inst.engine
                        nop.sync_info = mybir.SyncInfo(on_wait=[w], on_update=[])
                        new_insts.append(nop)
                        inserted += 1
                    si.on_wait = keep
                    changed = True
                new_insts.append(inst)
            if changed:
                bb.instructions.clear()
                for i in new_insts:
                    bb.instructions.append(i)
    return inserted


class _TC(tile.TileContext):
    def _drain_and_barrier(self, tick_clock, wait_clock):
        drain_inst = self.nc.sync.drain()
        wait_clock.add_sem_waits(
            drain_inst.ins, ScopedClock({None: tick_clock.global_clock})
        )
        si = drain_inst.ins.sync_info
        waits = list(si.on_wait or []) if si is not None else []
        if len(waits) > 1:
            si.on_wait = [waits[0]]
            for w in waits[1:]:
                d = self.nc.sync.drain()
                dsi = d.ins.sync_info
                if dsi is None:
                    d.ins.sync_info = mybir.SyncInfo(on_wait=[w], on_update=[])
                else:
                    dsi.on_wait = [w]
        self.nc.all_engine_barrier()
        assert self.sems is not None
        popped = self.nc._tile_sem_poison_stack.pop()
        assert popped is self._sem_poison
        self.nc.clear_and_free_semaphores(list(self.sems.allocated().values()))
        self.nc.all_engine_barrier()


# ---------------------------------------------------------------------------
# device program (identical on all 8 cores; only input data differs)
# ---------------------------------------------------------------------------
def _build_nc(repeat=1):
    nc = bass.Bass("TRN2", target_bir_lowering=False, debug=False,
                   num_devices=NCORES)
    xt = nc.dram_tensor("xt", [D, T], BF16, kind="ExternalInput").ap()
    wqm = nc.dram_tensor("wqm", [D, P], BF16, kind="ExternalInput").ap()
    wqb = nc.dram_tensor("wqb", [P, 1], F32, kind="ExternalInput").ap()
    wkm = nc.dram_tensor("wkm", [D, P], BF16, kind="ExternalInput").ap()
    wkb = nc.dram_tensor("wkb", [P, 1], F32, kind="ExternalInput").ap()
    wvm = nc.dram_tensor("wvm", [D, P], BF16, kind="ExternalInput").ap()
    wvb = nc.dram_tensor("wvb", [P, 1], F32, kind="ExternalInput").ap()
    wot = nc.dram_tensor("wo", [P, D], BF16, kind="ExternalInput").ap()
    yt = nc.dram_tensor("yt", [D, T], BF16, kind="ExternalOutput").ap()

    with _TC(nc) as tc, nc.allow_low_precision(
            reason="float32r is 32-bit; PE rounds internally"):
        _emit(nc, tc, xt, wqm, wqb, wkm, wkb, wvm, wvb, wot, yt,
              repeat=repeat)
    n = _legalize_waits(nc)
    return nc, n


def _emit(nc, tc, xt, wqm, wqb, wkm, wkb, wvm, wvb, wot, yt, repeat=1):
    ctxs = []

    def pool(name, bufs, space="SBUF"):
        p = tc.tile_pool(name=name, bufs=bufs, space=space)
        ctxs.append(p)
        return p.__enter__()

    wpool = pool("w", 1)
    persist = pool("persist", 1)
    xpool = pool("x", 2)
    epool = pool("e", 3)
    scrpool = pool("scr", 2)
    orawpool = pool("oraw", 2)
    yspool = pool("ys", 8)
    stgpool = pool("stg", 4)
    psum = pool("ps", 1, space="PSUM")

    def ps_tile(shape, tag, bufs, name):
        return psum.tile(shape, F32, tag=tag, bufs=bufs, name=name)

    # ---- constants / weights (loaded once) ----
    wq_sb = wpool.tile([P, KC, P], BF16)
    wk_sb = wpool.tile([P, KC, P], BF16)
    wv_sb = wpool.tile([P, KC, P], BF16)
    nc.sync.dma_start(wq_sb[:], wqm.rearrange("(k p) d -> p k d", p=P))
    nc.sync.dma_start(wk_sb[:], wkm.rearrange("(k p) d -> p k d", p=P))
    nc.sync.dma_start(wv_sb[:], wvm.rearrange("(k p) d -> p k d", p=P))
    wqb_sb = wpool.tile([P, 1], F32)
    wkb_sb = wpool.tile([P, 1], F32)
    wvb_sb = wpool.tile([P, 1], F32)
    nc.sync.dma_start(wqb_sb[:], wqb[:, :])
    nc.sync.dma_start(wkb_sb[:], wkb[:, :])
    nc.sync.dma_start(wvb_sb[:], wvb[:, :])
    wo_sb = wpool.tile([P, D], BF16)
    nc.sync.dma_start(wo_sb[:], wot[:, :])
    # memset can't write float32r; memset f32 then DVE-copy (which rounds)
    ones_f32 = wpool.tile([P, TCH], F32)
    nc.vector.memset(ones_f32[:], 1.0)
    ones_sb = wpool.tile([1, TCH], BF16)
    nc.vector.tensor_copy(ones_sb[:], ones_f32[0:1, :])
    ident = wpool.tile([P, P], F32)
    make_identity(nc, ident[:])
    ident_r = wpool.tile([P, P], BF16)
    nc.vector.tensor_copy(ident_r[:], ident[:])

    # ---- persistent activations ----
    qT = persist.tile([P, T], BF16)
    kT = persist.tile([P, T], BF16)
    v_comb = persist.tile([P, T // P, VW], BF16)    # [tok%128, tok-tile, 130]
    nc.vector.tensor_copy(
        v_comb[:, :, DK:DK + 1],
        ones_f32[:, 0:1].broadcast_to([P, T // P, 1]))
    nc.vector.tensor_copy(
        v_comb[:, :, VW - 1:VW],
        ones_f32[:, 0:1].broadcast_to([P, T // P, 1]))

    NB = B * repeat
    if ATTN_ONLY:
        NB = repeat  # groups over batch 0 only, no proj of later batches

    # ---------------- projection of batch bg1 (mod B), chunk c -------------
    def make_proj_units(bg1, c):
        b1 = bg1 % B
        c0 = b1 * S + c * TCH
        h = {}
        units = []

        def u_load():
            x_ch = xpool.tile([P, KC, TCH], BF16, tag="x", name="x_ch")
            nc.sync.dma_start(
                x_ch[:], xt[:, c0:c0 + TCH].rearrange("(k p) n -> p k n", p=P))
            h["x"] = x_ch
        units.append(u_load)

        def mk_pass(w_sb, drain_fn, key):
            def u_start():
                h[key] = ps_tile([P, TCH], "p", 1, f"prj_{key}")
                nc.tensor.matmul(h[key][:], w_sb[:, 0, :], h["x"][:, 0, :],
                                 start=True, stop=False)
                nc.tensor.matmul(h[key][:], w_sb[:, 1, :], h["x"][:, 1, :],
                                 start=False, stop=False)
            units.append(u_start)
            for k0 in (2, 4, 6):
                def u_mid(k0=k0):
                    nc.tensor.matmul(h[key][:], w_sb[:, k0, :],
                                     h["x"][:, k0, :], start=False, stop=False)
                    nc.tensor.matmul(h[key][:], w_sb[:, k0 + 1, :],
                                     h["x"][:, k0 + 1, :],
                                     start=False, stop=(k0 == 6))
                units.append(u_mid)
            units.append(drain_fn)

        def drain_q():
            nc.vector.tensor_scalar_add(qT[:, c0:c0 + TCH], h["q"][:],
                                        wqb_sb[:, 0:1])
        def drain_k():
            nc.vector.tensor_scalar_add(kT[:, c0:c0 + TCH], h["k"][:],
                                        wkb_sb[:, 0:1])
        def drain_v():
            v_scr = scrpool.tile([P, TCH], BF16, tag="vscr", name="v_scr")
            nc.vector.tensor_scalar_add(v_scr[:], h["v"][:], wvb_sb[:, 0:1])
            h["vs"] = v_scr

        mk_pass(wq_sb, drain_q, "q")
        mk_pass(wk_sb, drain_k, "k")
        mk_pass(wv_sb, drain_v, "v")

        for tt in range(TCH // P):
            def u_tr(tt=tt):
                vt = (c0 // P) + tt
                tr = psum.tile([P, TCH], BF16, tag="y", bufs=1, name="tr")
                nc.tensor.transpose(tr[:, 0:P],
                                    h["vs"][:, tt * P:(tt + 1) * P], ident_r[:])
                nc.vector.tensor_copy(v_comb[:, vt, 0:DK], tr[:, 0:DK])
                nc.vector.tensor_copy(v_comb[:, vt, DK + 1:2 * DK + 1],
                                      tr[:, DK:2 * DK])
            units.append(u_tr)
        return units

    # -------- normalization + out-projection of a finished group ----------
    def make_norm_outproj_units(prev, noproj):
        b, qc, ps_oA, ps_oB = prev
        t0 = b * S
        q0 = t0 + qc * TCH
        h = {}
        units = []

        def u_recip():
            stgA = stgpool.tile([1, TCH], BF16, tag="stg", name="stgA")
            stgB = stgpool.tile([1, TCH], BF16, tag="stg", name="stgB")
            nc.vector.reciprocal(stgA[:], ps_oA[DK:DK + 1, :])
            nc.vector.reciprocal(stgB[:], ps_oB[DK:DK + 1, :])
            h["rA"], h["rB"] = stgA, stgB
        units.append(u_recip)

        def u_bc():
            bc = ps_tile([P, 2 * TCH], "s", 2, "bc")
            nc.tensor.matmul(bc[0:DK, 0:TCH], ones_sb[0:1, 0:DK], h["rA"][:],
                             start=True, stop=True)
            nc.tensor.matmul(bc[0:DK, TCH:2 * TCH], ones_sb[0:1, 0:DK],
                             h["rB"][:], start=True, stop=True)
            h["bcA"] = bc[0:DK, 0:TCH]
            h["bcB"] = bc[0:DK, TCH:2 * TCH]
        units.append(u_bc)

        def u_cpA():
            oraw = orawpool.tile([P, TCH], BF16, tag="oraw", name="oraw")
            nc.vector.tensor_copy(oraw[0:DK, :], ps_oA[0:DK, :])
            h["o"] = oraw
        units.append(u_cpA)

        def u_cpB():
            nc.vector.tensor_copy(h["o"][DK:P, :], ps_oB[0:DK, :])
        units.append(u_cpB)

        def u_mulA():
            nc.vector.tensor_mul(h["o"][0:DK, :], h["o"][0:DK, :],
                                 h["bcA"])
        units.append(u_mulA)

        def u_mulB():
            nc.vector.tensor_mul(h["o"][DK:P, :], h["o"][DK:P, :],
                                 h["bcB"])
        units.append(u_mulB)

        for m in range(KC):
            def u_out(m=m):
                tag = ("y", "p")[m % 2] if noproj else "y"
                ps_y = ps_tile([P, TCH], tag, 1, "ps_y")
                nc.tensor.matmul(ps_y[:], wo_sb[:, m * P:(m + 1) * P],
                                 h["o"][:], start=True, stop=True)
                ys = yspool.tile([P, TCH], BF16, tag="yst", name="ys")
                nc.vector.tensor_copy(ys[:], ps_y[:])
                nc.gpsimd.dma_start(yt[m * P:(m + 1) * P, q0:q0 + TCH], ys[:])
            units.append(u_out)
        return units

    # ---------------- main pipelined schedule ------------------------------
    # prologue: projections of batch 0 (all 4 chunks, un-interleaved)
    for c in range(NQC):
        for u in make_proj_units(0, c):
            u()

    prev = None
    for bg in range(NB):
        b = 0 if ATTN_ONLY else bg % B
        t0 = b * S
        for qc in range(NQC):
            q0 = t0 + qc * TCH
            extras = []
            if not ATTN_ONLY:
                if prev is not None:
                    extras += make_norm_outproj_units(prev,
                                                      noproj=(bg + 1 >= NB))
                if bg + 1 < NB:
                    extras += make_proj_units(bg + 1, qc)

            ps_oA = ps_tile([DK + 1, TCH], "o", 2, "ps_oA")
            ps_oB = ps_tile([DK + 1, TCH], "o", 2, "ps_oB")

            def attnv(kc, e_t):
                vt = (t0 // P) + kc
                nc.tensor.matmul(ps_oA[:], v_comb[:, vt, 0:DK + 1],
                                 e_t[:, 0:TCH],
                                 start=(kc == 0), stop=(kc == NKT - 1))
                nc.tensor.matmul(ps_oB[:], v_comb[:, vt, DK + 1:VW],
                                 e_t[:, TCH:2 * TCH],
                                 start=(kc == 0), stop=(kc == NKT - 1))

            pending = None
            for kc in range(NKT):
                kt0 = t0 + kc * P
                s_t = ps_tile([P, 2 * TCH], "s", 2, "s_t")
                nc.tensor.matmul(s_t[:, 0:TCH], kT[0:DK, kt0:kt0 + P],
                                 qT[0:DK, q0:q0 + TCH], start=True, stop=True)
                nc.tensor.matmul(s_t[:, TCH:2 * TCH], kT[DK:P, kt0:kt0 + P],
                                 qT[DK:P, q0:q0 + TCH], start=True, stop=True)
                e_t = epool.tile([P, 2 * TCH], BF16, tag="e", name="e_t")
                nc.scalar.activation(e_t[:], s_t[:], EXPF, scale=0.125)
                if pending is not None:
                    attnv(*pending)
                pending = (kc, e_t)
                n_pop = math.ceil(len(extras) / (NKT - kc))
                for _ in range(min(n_pop, len(extras))):
                    extras.pop(0)()
            attnv(*pending)
            for u in extras:
                u()
            prev = (b, qc, ps_oA, ps_oB)

    if not ATTN_ONLY:
        for u in make_norm_outproj_units(prev, noproj=True):
            u()

    for p in reversed(ctxs):
        p.__exit__(None, None, None)


_CACHED = {}


def _get_nc(repeat=1):
    if repeat not in _CACHED:
        _CACHED[repeat] = _build_nc(repeat=repeat)[0]
    return _CACHED[repeat]


def _make_in_maps(x, wq, bq, wk, bk, wv, bv, wo, bo):
    from ml_dtypes import bfloat16
    x = np.asarray(x, np.float32)
    wq, bq = np.asarray(wq, bfloat16), np.asarray(bq, np.float32)
    wk, bk = np.asarray(wk, bfloat16), np.asarray(bk, np.float32)
    wv, bv = np.asarray(wv, bfloat16), np.asarray(bv, np.float32)
    wo = np.asarray(wo, bfloat16)
    xT = np.ascontiguousarray(x.reshape(T, D).T).astype(bfloat16)
    maps = []
    for c in range(NCORES):
        sl = slice(c * P, (c + 1) * P)
        maps.append({
            "xt": xT,
            "wqm": np.ascontiguousarray(wq[:, sl]),
            "wqb": np.ascontiguousarray(bq[sl])[:, None],
            "wkm": np.ascontiguousarray(wk[:, sl]),
            "wkb": np.ascontiguousarray(bk[sl])[:, None],
            "wvm": np.ascontiguousarray(wv[:, sl]),
            "wvb": np.ascontiguousarray(bv[sl])[:, None],
            "wo": np.ascontiguousarray(wo[sl, :]),
        })
    return maps


def kernel(x, wq, bq, wk, bk, wv, bv, wo, bo):
    nc = _get_nc()
    in_maps = _make_in_maps(x, wq, bq, wk, bk, wv, bv, wo, bo)
    res = run_bass_kernel_spmd(nc, in_maps, core_ids=list(range(NCORES)),
                               trace=False)
    yT = res.results[0]["yt"].astype(np.float32)
    for c in range(1, NCORES):
        yT += res.results[c]["yt"].astype(np.float32)
    y = np.ascontiguousarray(yT.T).reshape(B, S, D)
    return y + np.asarray(bo, np.float32)[None, None, :]
